# revision 1
# baseline (speedup 1.0000x reference)
"""MoD (mixture-of-depths) attention Bass kernel for Trainium2, 8 NeuronCores.

Problem: B=4, L=4096, D=1024, H=16, HD=64, K=1024 (top-25% tokens per row).
  scores = x @ w_router + b ; idx = top_k(scores, 1024) (desc order)
  xs = x[idx]; causal attention over score-ordered subsequence; out = x with
  selected rows replaced by attention output.

Sharding: core pair (2b, 2b+1) handles batch row b. Within a pair the 16
heads are split 8/8 (tensor parallel). Each core computes half the router
scores (AllGather within pair), full top-k redundantly, gathers xs via
indirect DMA, computes QKV for its 8 heads, causal attention, and a partial
out-projection over its 512 e-dims; a pair ReduceScatter(add) yields each
core's 512-token half of the final [1024, 1024] attention output.
Host reassembles: out[b] = x[b].copy(); out[b][idx] = concat(halves).
"""

import numpy as np

import concourse.bass as bass
import concourse.bacc as bacc
import concourse.mybir as mybir
import concourse.tile as tile
from concourse import library_config
from concourse.tile_rust import add_dep_helper
from concourse.bass import IndirectOffsetOnAxis
from concourse.bass_utils import run_bass_kernel_spmd
from concourse.library_overlay import lower_extended_insts

F32 = mybir.dt.float32
I32 = mybir.dt.int32
AF = mybir.ActivationFunctionType
OP = mybir.AluOpType

B, L, D = 4, 4096, 1024
H, HD = 16, 64
K = 1024
SCALE = 1.0 / 8.0
NEG = -1.0e30
EH = 512          # e-dims per core (8 heads)
NH_OWN = 8        # heads per core
N_TC = 8          # token chunks of 128 (K = 1024)
N_EBLK = 4        # e-blocks of 128 per core


def _consts():
    """Inline constant tensors (baked into the NEFF, DMA'd at load time)."""
    c = {}
    c["identity"] = np.eye(128, dtype=np.float32)
    # causal mask accumulators for S^T tiles [tk=128, tq=512]:
    # tile (m, n) crossing has delta = m*128 - n*512 in {0,128,256,384};
    # invalid (tk > tq) iff p + delta > f  -> add -1e30 there.
    u = np.zeros((4, 128, 512), dtype=np.float32)
    p = np.arange(128)[:, None]
    f = np.arange(512)[None, :]
    for di in range(4):
        u[di] = np.where(p + di * 128 > f, NEG, 0.0).astype(np.float32)
    c["umask"] = u
    # compaction index iota over [16, 256]: value = p*256 + f (fp32)
    c["iota16"] = (np.arange(16)[:, None] * 256 + np.arange(256)[None, :]).astype(
        np.float32
    )
    c["negones16"] = np.full((16, 256), -1.0, dtype=np.float32)
    # rank decomposition consts
    c["cbase"] = np.tile((np.arange(8) * 128).astype(np.float32), (128, 1))
    c["cbase_hi"] = c["cbase"] + 128.0
    c["cidx"] = np.tile(np.arange(8).astype(np.float32), (128, 1))
    c["pcol"] = np.tile(np.arange(128).astype(np.float32), (128, 1))
    return c


def _quantile_for(n_valid, k_adj):
    # kth_largest: k_adj = floor(omq * (n_valid - 1) / 2^32); out[1] = desc[k_adj + 1]
    return 1.0 - (k_adj + 0.5) / (n_valid - 1)


def build_program(n_cores=8, percore_shapes=False):
    """Builds the SPMD Bass program (same program on all cores; per-core
    behavior comes only from per-core input data). n_cores=1 builds the
    collective-free single-core variant (for simulation): full scores on the
    one core and no ReduceScatter (ypart is the output)."""
    spmd = n_cores > 1
    L_OWN = L // 2 if (spmd or percore_shapes) else L
    NSC = L_OWN // 128  # score tiles

    nc = bacc.Bacc("TRN2", num_devices=n_cores, debug=False)

    # ---- I/O ----
    x_row = nc.dram_tensor("x_row", [L, D], F32, kind="ExternalInput")
    x_sc = nc.dram_tensor("x_sc", [L_OWN, D], F32, kind="ExternalInput")
    w_rt = nc.dram_tensor("w_rt", [1, D], F32, kind="ExternalInput")
    b_rt = nc.dram_tensor("b_rt", [1, 1], F32, kind="ExternalInput")
    # wq/wk tiled host-side: [eblk, dblk, 128d, 128e]
    wq_t = nc.dram_tensor("wq_t", [N_EBLK, 8, 128, 128], F32, kind="ExternalInput")
    wk_t = nc.dram_tensor("wk_t", [N_EBLK, 8, 128, 128], F32, kind="ExternalInput")
    wv_o = nc.dram_tensor("wv_o", [D, EH], F32, kind="ExternalInput")
    wo_o = nc.dram_tensor("wo_o", [EH, D], F32, kind="ExternalInput")

    y_out_rows = K // 2 if (spmd or percore_shapes) else K
    y_out = nc.dram_tensor("y_out", [y_out_rows, D], F32, kind="ExternalOutput")
    idx_out = nc.dram_tensor("idx_out", [K], I32, kind="ExternalOutput")

    # ---- internal DRAM ----
    s_half_d = nc.dram_tensor("s_half_d", [L_OWN], F32, kind="Internal")
    if spmd or percore_shapes:
        s_full_d = nc.dram_tensor("s_full_d", [L], F32, kind="Internal")
        ypart_d = nc.dram_tensor("ypart_d", [K, D], F32, kind="Internal")
        y_red_d = nc.dram_tensor("y_red_d", [K // 2, D], F32, kind="Internal")
    else:
        s_full_d = s_half_d
    flat_v_d = nc.dram_tensor("flat_v_d", [1, K], F32, kind="Internal")
    rden_d = nc.dram_tensor("rden_d", [16, 512], F32, kind="Internal")

    consts = {k: nc.inline_tensor(v, name=f"c_{k}") for k, v in _consts().items()}

    PAIRS = [[2 * i, 2 * i + 1] for i in range(max(n_cores // 2, 1))]

    with tile.TileContext(nc) as tc:
        with (
            tc.tile_pool(name="const", bufs=1) as cpool,
            tc.tile_pool(name="ps", bufs=4, space="PSUM") as psp,
            tc.tile_pool(name="pst", bufs=2, space="PSUM") as pstp,
            tc.tile_pool(name="psb", bufs=2, space="PSUM") as psb,
        ):
            # ---------- constants to SBUF ----------
            ident = cpool.tile([128, 128], F32)
            nc.sync.dma_start(ident[:], consts["identity"][:])
            umask = cpool.tile([128, 4, 512], F32)
            for di in range(4):
                nc.sync.dma_start(umask[:, di, :], consts["umask"][di])
            cbase = cpool.tile([128, 8], F32)
            nc.sync.dma_start(cbase[:], consts["cbase"][:])
            cbase_hi = cpool.tile([128, 8], F32)
            nc.sync.dma_start(cbase_hi[:], consts["cbase_hi"][:])
            cidx = cpool.tile([128, 8], F32)
            nc.sync.dma_start(cidx[:], consts["cidx"][:])
            pcol = cpool.tile([128, 128], F32)
            nc.sync.dma_start(pcol[:], consts["pcol"][:])

            # phase-scoped activation tensors (manual release in sequence)
            actp1 = tc.alloc_tile_pool(name="actp1", bufs=1)
            xsT = actp1.tile([128, 8, K], F32, tag="xsT")
            i_sort_i = cpool.tile([128, 8], I32, tag="isrt")

            # ---------- phase A: scores, top-k, gather, transposes ----------
            with (
                tc.tile_pool(name="sa", bufs=1) as spool,
                tc.tile_pool(name="sca", bufs=2) as scpool,
            ):
                iota16 = spool.tile([16, 256], F32)
                nc.sync.dma_start(iota16[:], consts["iota16"][:])
                neg16 = spool.tile([16, 256], F32)
                nc.sync.dma_start(neg16[:], consts["negones16"][:])
                w_rep = spool.tile([128, D], F32)
                nc.sync.dma_start(w_rep[:], w_rt[:].to_broadcast((128, D)))
                b_bc = spool.tile([128, 1], F32)
                nc.sync.dma_start(b_bc[:], b_rt[:].to_broadcast((128, 1)))

                # scores (own half): tile j holds x rows {p*NSC + j} so the
                # score vector lands p-major => contiguous DRAM store.
                s_half = spool.tile([128, NSC], F32)
                x_sc_v = x_sc[:].rearrange("(p j) d -> j p d", j=NSC)
                for j in range(NSC):
                    xt = scpool.tile([128, D], F32, tag="xsc")
                    nc.sync.dma_start(xt[:], x_sc_v[j])
                    prod = scpool.tile([128, D], F32, tag="prod")
                    nc.vector.tensor_tensor(
                        out=prod[:], in0=xt[:], in1=w_rep[:], op=OP.mult
                    )
                    acc_scr = scpool.tile([128, D], F32, tag="accscr")
                    nc.scalar.activation(
                        acc_scr[:], prod[:], AF.Copy,
                        accum_out=s_half[:, j : j + 1],
                    )
                nc.vector.tensor_scalar(
                    s_half[:], s_half[:], b_bc[:], None, op0=OP.add
                )
                nc.sync.dma_start(
                    s_half_d[:].rearrange("(p j) -> p j", j=NSC), s_half[:]
                )

                # all-gather scores within pair
                if percore_shapes:
                    nc.sync.dma_start(s_full_d[0:L_OWN], s_half_d[:])
                    nc.sync.dma_start(s_full_d[L_OWN:L], s_half_d[:])
                if spmd:
                    nc.gpsimd.collective_compute(
                        "AllGather",
                        OP.bypass,
                        replica_groups=PAIRS,
                        ins=[s_half_d[:]],
                        outs=[s_full_d[:]],
                    )

                s_sb = spool.tile([128, 32], F32)
                nc.sync.dma_start(
                    s_sb[:], s_full_d[:].rearrange("(p f) -> p f", f=32)
                )
                s16 = spool.tile([16, 256], F32)
                nc.sync.dma_start(
                    s16[:], s_full_d[:].rearrange("(p f) -> p f", f=256)
                )

                # exact threshold T (1024th largest) via 3 masked rounds
                s_work = spool.tile([128, 32], F32)
                nc.vector.tensor_copy(s_work[:], s_sb[:])
                negtile = spool.tile([128, 32], F32)
                nc.vector.memset(negtile[:], NEG)
                kth = spool.tile([1, 2], F32)
                t_bc = spool.tile([128, 1], F32)
                rounds = [(4096, 508), (3586, 508), (3076, 2)]
                for r, (n_valid, k_adj) in enumerate(rounds):
                    nc.gpsimd.kth_largest(
                        kth[:], s_work[:], 32, 510,
                        quantile=_quantile_for(n_valid, k_adj),
                    )
                    nc.gpsimd.partition_broadcast(t_bc[:], kth[0:1, 1:2])
                    if r < len(rounds) - 1:
                        ge = spool.tile([128, 32], mybir.dt.uint8, tag="gemask")
                        nc.vector.tensor_scalar(
                            ge[:], s_work[:], t_bc[:], None, op0=OP.is_ge
                        )
                        nc.vector.copy_predicated(s_work[:], ge[:], negtile[:])

                # compact selected values & original indices
                shifted = spool.tile([16, 256], F32)
                nc.vector.tensor_scalar(
                    shifted[:], s16[:], t_bc[0:16, :], None, op0=OP.subtract
                )
                mask16 = spool.tile([16, 256], mybir.dt.uint8)
                nc.vector.tensor_scalar(
                    mask16[:], shifted[:], 0.0, None, op0=OP.is_ge
                )
                idx16 = spool.tile([16, 256], F32)
                nc.vector.tensor_copy(idx16[:], neg16[:])
                nc.vector.copy_predicated(idx16[:], mask16[:], iota16[:])

                nf = spool.tile([1, 1], mybir.dt.uint32)
                v_comp = spool.tile([16, 64], F32)
                nc.gpsimd.sparse_gather(v_comp[:], shifted[:], num_found=nf[:])
                nf2 = spool.tile([1, 1], mybir.dt.uint32)
                i_comp = spool.tile([16, 64], F32)
                nc.gpsimd.sparse_gather(i_comp[:], idx16[:], num_found=nf2[:])

                # reshape compacted streams: [16, 64] -> [64, 16] -> [8, 128]
                v64_ps = psb.tile([64, 16], F32, tag="tsm")
                nc.tensor.transpose(v64_ps[:], v_comp[:], ident[0:16, 0:16])
                v64 = spool.tile([64, 16], F32)
                nc.vector.tensor_copy(v64[:], v64_ps[:])
                i64_ps = psb.tile([64, 16], F32, tag="tsm")
                nc.tensor.transpose(i64_ps[:], i_comp[:], ident[0:16, 0:16])
                i64 = spool.tile([64, 16], F32)
                nc.vector.tensor_copy(i64[:], i64_ps[:])

                s8v = spool.tile([8, 128], F32)
                nc.sync.dma_start(s8v[:], v64[:])
                s8i = spool.tile([8, 128], F32)
                nc.sync.dma_start(s8i[:], i64[:])

                # flat [1, 1024] via DRAM, then broadcast to [128, 1024]
                nc.sync.dma_start(flat_v_d[:], s8v[:])
                rep = spool.tile([128, K], F32)
                nc.sync.dma_start(rep[:], flat_v_d[:].to_broadcast((128, K)))

                # per-chunk scalars [128, 8]
                vch_ps = psb.tile([128, 8], F32, tag="tsm")
                nc.tensor.transpose(vch_ps[:], s8v[:], ident[0:8, 0:8])
                v_ch = spool.tile([128, 8], F32)
                nc.vector.tensor_copy(v_ch[:], vch_ps[:])
                ich_ps = psb.tile([128, 8], F32, tag="tsm")
                nc.tensor.transpose(ich_ps[:], s8i[:], ident[0:8, 0:8])
                i_ch = spool.tile([128, 8], F32)
                nc.vector.tensor_copy(i_ch[:], ich_ps[:])

                # ranks among selected
                ranks = spool.tile([128, 8], F32)
                for c in range(8):
                    rankscr = scpool.tile([128, K], F32, tag="rankscr")
                    nc.vector.tensor_tensor(
                        out=rankscr[:], in0=rep[:],
                        in1=v_ch[:, c : c + 1].to_broadcast((128, K)),
                        op=OP.is_gt,
                    )
                    rankscr2 = scpool.tile([128, K], F32, tag="rankscr2")
                    nc.scalar.activation(
                        rankscr2[:], rankscr[:], AF.Copy,
                        accum_out=ranks[:, c : c + 1],
                    )

                # permutation: i_sorted[p, c] = original idx with rank c*128+p
                isort_ps = psb.tile([128, 8], F32, tag="tsm")
                for c in range(8):
                    rank_c = ranks[:, c : c + 1]
                    ge_lo = spool.tile([128, 8], F32, tag="rge")
                    nc.vector.tensor_tensor(
                        out=ge_lo[:], in0=rank_c.to_broadcast((128, 8)),
                        in1=cbase[:], op=OP.is_ge,
                    )
                    lt_hi = spool.tile([128, 8], F32, tag="rlt")
                    nc.vector.tensor_tensor(
                        out=lt_hi[:], in0=rank_c.to_broadcast((128, 8)),
                        in1=cbase_hi[:], op=OP.is_lt,
                    )
                    r_ci = spool.tile([128, 8], F32, tag="rci")
                    nc.vector.tensor_tensor(
                        out=r_ci[:], in0=ge_lo[:], in1=lt_hi[:], op=OP.mult
                    )
                    cdiv = spool.tile([128, 1], F32, tag="cdiv")
                    tmp8 = spool.tile([128, 8], F32, tag="tmp8")
                    nc.vector.tensor_tensor(
                        out=tmp8[:], in0=r_ci[:], in1=cidx[:], op=OP.mult
                    )
                    nc.vector.reduce_sum(
                        cdiv[:], tmp8[:], axis=mybir.AxisListType.X
                    )
                    rmod = spool.tile([128, 1], F32, tag="rmod")
                    nc.vector.scalar_tensor_tensor(
                        out=rmod[:], in0=cdiv[:], scalar=-128.0, in1=rank_c,
                        op0=OP.mult, op1=OP.add,
                    )
                    m_oh = scpool.tile([128, 128], F32, tag="moh")
                    nc.vector.tensor_tensor(
                        out=m_oh[:], in0=rmod[:].to_broadcast((128, 128)),
                        in1=pcol[:], op=OP.is_equal,
                    )
                    m_ci = scpool.tile([128, 128], F32, tag="mci")
                    nc.vector.tensor_scalar(
                        m_ci[:], m_oh[:], i_ch[:, c : c + 1], None, op0=OP.mult
                    )
                    nc.tensor.matmul(
                        isort_ps[:], m_ci[:], r_ci[:],
                        start=(c == 0), stop=(c == 7),
                    )

                nc.vector.tensor_copy(i_sort_i[:], isort_ps[:])
                i_sort_f = spool.tile([128, 8], F32)
                nc.vector.tensor_copy(i_sort_f[:], isort_ps[:])

                # idx_out [1024] in token order (t = c*128 + p)
                it_ps = psb.tile([8, 128], F32, tag="tsm")
                nc.tensor.transpose(it_ps[:], i_sort_f[:], ident[:])
                it_sb = spool.tile([8, 128], I32)
                nc.vector.tensor_copy(it_sb[:], it_ps[:])
                nc.sync.dma_start(
                    idx_out[:].rearrange("(c p) -> c p", p=128), it_sb[:]
                )

                # gather xs [128, 8, 1024] (t = c*128 + p), then transpose
                xs = spool.tile([128, N_TC, D], F32, tag="xs")
                for c in range(N_TC):
                    nc.gpsimd.indirect_dma_start(
                        out=xs[:, c, :],
                        out_offset=None,
                        in_=x_row[:],
                        in_offset=IndirectOffsetOnAxis(
                            ap=i_sort_i[:, c : c + 1], axis=0
                        ),
                    )
                for dblk in range(8):
                    for c in range(N_TC):
                        tp = pstp.tile([128, 128], F32, tag="ps128")
                        nc.tensor.transpose(
                            tp[:], xs[:, c, dblk * 128 : (dblk + 1) * 128],
                            ident[:],
                        )
                        nc.any.tensor_copy(
                            xsT[:, dblk, c * 128 : (c + 1) * 128], tp[:]
                        )

            # ---------- Q^T, K^T [eblk][128e, 1024t]; V [tc][128t, 8h, 65] ----------
            actp2 = tc.alloc_tile_pool(name="actp2", bufs=1)
            qT = actp2.tile([128, N_EBLK, K], F32, tag="qT")
            kT = actp2.tile([128, N_EBLK, K], F32, tag="kT")
            v_sb = actp2.tile([128, N_TC, NH_OWN, 65], F32, tag="v")
            wpool = tc.alloc_tile_pool(name="wpool", bufs=2)
            for eblk in range(N_EBLK):
                wq_sb = wpool.tile([128, 8, 128], F32, tag="wq")
                nc.sync.dma_start(wq_sb[:], wq_t[eblk].rearrange("k p e -> p k e"))
                wk_sb = wpool.tile([128, 8, 128], F32, tag="wk")
                nc.sync.dma_start(wk_sb[:], wk_t[eblk].rearrange("k p e -> p k e"))
                for tch in range(2):
                    tsl = bass.ts(tch, 512)
                    pq = psp.tile([128, 512], F32, tag="ps512")
                    pk = psp.tile([128, 512], F32, tag="ps512")
                    for dblk in range(8):
                        nc.tensor.matmul(
                            pq[:], wq_sb[:, dblk, :], xsT[:, dblk, tsl],
                            start=(dblk == 0), stop=(dblk == 7),
                        )
                    for dblk in range(8):
                        nc.tensor.matmul(
                            pk[:], wk_sb[:, dblk, :], xsT[:, dblk, tsl],
                            start=(dblk == 0), stop=(dblk == 7),
                        )
                    nc.any.tensor_copy(qT[:, eblk, tsl], pq[:])
                    nc.any.tensor_copy(kT[:, eblk, tsl], pk[:])

            wpool.release()
            wvp = tc.alloc_tile_pool(name="wvp", bufs=1)
            wv_all = wvp.tile([128, 8, 512], F32, tag="wv")
            nc.sync.dma_start(
                wv_all[:], wv_o[:].rearrange("(k p) e -> p k e", p=128)
            )
            v_one = wvp.tile([128, N_TC * NH_OWN], F32, tag="vone")
            nc.vector.memset(v_one[:], 1.0)
            nc.vector.tensor_copy(
                v_sb[:, :, :, 64],
                v_one[:].rearrange("p (t h) -> p t h", t=N_TC),
            )
            for tc_i in range(N_TC):
                pv = psp.tile([128, 512], F32, tag="ps512")
                for dblk in range(8):
                    nc.tensor.matmul(
                        pv[:],
                        xsT[:, dblk, tc_i * 128 : (tc_i + 1) * 128],
                        wv_all[:, dblk, :],
                        start=(dblk == 0), stop=(dblk == 7),
                    )
                nc.any.tensor_copy(
                    v_sb[:, tc_i, :, 0:64],
                    pv[:].rearrange("p (h e) -> p h e", h=8),
                )

            # ---------- attention per head; O^T rows hh*64..hh*64+63 ----------
            wvp.release()
            actp3 = tc.alloc_tile_pool(name="actp3", bufs=1)
            oT = actp3.tile([128, N_EBLK, K], F32, tag="oT")
            expp = tc.alloc_tile_pool(name="expp", bufs=3)
            for eblk in range(N_EBLK):
                for sub in range(2):
                    hh = eblk * 2 + sub
                    esl = slice(sub * 64, sub * 64 + 64)
                    for n in range(2):
                        tql = bass.ts(n, 512)
                        po = psp.tile([65, 512], F32, tag="ps512")
                        n_m = 4 * n + 4
                        for m in range(n_m):
                            ps_s = psp.tile([128, 512], F32, tag="ps512")
                            crossing = m * 128 + 127 > n * 512
                            if crossing:
                                di = m - 4 * n
                                nc.tensor.matmul(
                                    ps_s[:], ident[:], umask[:, di, :],
                                    start=True, stop=False,
                                )
                            nc.tensor.matmul(
                                ps_s[:],
                                kT[esl, eblk, m * 128 : (m + 1) * 128],
                                qT[esl, eblk, tql],
                                start=not crossing, stop=True,
                                tile_position=(sub * 64, 0),
                            )
                            es = expp.tile([128, 512], F32, tag="es")
                            nc.scalar.activation(
                                es[:], ps_s[:], AF.Exp, scale=SCALE
                            )
                            nc.tensor.matmul(
                                po[:], v_sb[:, m, hh, :], es[:],
                                start=(m == 0), stop=(m == n_m - 1),
                            )
                        # normalize rows 0..63 by row 64
                        r_row = expp.tile([1, 512], F32, tag="rrow")
                        nc.vector.reciprocal(r_row[:], po[64:65, :])
                        slot = hh * 2 + n
                        nc.sync.dma_start(rden_d[slot : slot + 1, :], r_row[:])
                        r_bc = expp.tile([64, 512], F32, tag="rbc")
                        nc.sync.dma_start(
                            r_bc[:],
                            rden_d[slot : slot + 1, :].to_broadcast((64, 512)),
                        )
                        nc.vector.tensor_tensor(
                            out=oT[sub * 64 : sub * 64 + 64, eblk, tql],
                            in0=po[0:64, :], in1=r_bc[:], op=OP.mult,
                        )

            # ---------- out-projection partial: ypart[t, :] ----------
            wop = tc.alloc_tile_pool(name="wop", bufs=1)
            wo_all = wop.tile([128, N_EBLK, D], F32, tag="wo")
            nc.sync.dma_start(
                wo_all[:], wo_o[:].rearrange("(k p) d -> p k d", p=128)
            )
            ydst = ypart_d if (spmd or percore_shapes) else y_out
            for tc_i in range(N_TC):
                for dc in range(2):
                    py = psp.tile([128, 512], F32, tag="ps512")
                    for eblk in range(N_EBLK):
                        nc.tensor.matmul(
                            py[:],
                            oT[:, eblk, tc_i * 128 : (tc_i + 1) * 128],
                            wo_all[:, eblk, dc * 512 : (dc + 1) * 512],
                            start=(eblk == 0), stop=(eblk == N_EBLK - 1),
                        )
                    y_sb = expp.tile([128, 512], F32, tag="ysb")
                    nc.any.tensor_copy(y_sb[:], py[:])
                    nc.sync.dma_start(
                        ydst[tc_i * 128 : (tc_i + 1) * 128,
                             dc * 512 : (dc + 1) * 512],
                        y_sb[:],
                    )

            wop.release()
            expp.release()
            actp3.release()
            actp2.release()
            actp1.release()

            if percore_shapes:
                nc.sync.dma_start(y_out[:], ypart_d[0 : K // 2, :])
            if spmd:
                nc.gpsimd.collective_compute(
                    "ReduceScatter",
                    OP.add,
                    replica_groups=PAIRS,
                    ins=[ypart_d[:]],
                    outs=[y_red_d[:]],
                )
                nc.sync.dma_start(y_out[:], y_red_d[:])

    nc.compile()
    return nc


_NC_CACHE = {}


def _get_nc(n_cores=8):
    if n_cores not in _NC_CACHE:
        _NC_CACHE[n_cores] = build_program(n_cores)
    return _NC_CACHE[n_cores]


def _weight_tiles(w_half):
    # [1024, 512] -> [eblk, dblk, 128d, 128e]
    return np.ascontiguousarray(
        w_half.reshape(8, 128, 4, 128).transpose(2, 0, 1, 3)
    )


def _build_in_maps(inputs):
    x = np.ascontiguousarray(np.asarray(inputs["x"], np.float32))
    w_router = np.asarray(inputs["w_router"], np.float32)
    b_router = np.asarray(inputs["b_router"], np.float32)
    wq = np.asarray(inputs["wq"], np.float32)
    wk = np.asarray(inputs["wk"], np.float32)
    wv = np.asarray(inputs["wv"], np.float32)
    wo = np.asarray(inputs["wo"], np.float32)

    in_maps = []
    for core in range(8):
        b = core // 2
        half = core % 2
        esl = slice(half * EH, (half + 1) * EH)
        in_maps.append(
            {
                "x_row": x[b],
                "x_sc": np.ascontiguousarray(x[b, half * 2048 : (half + 1) * 2048]),
                "w_rt": w_router.reshape(1, D),
                "b_rt": b_router.reshape(1, 1),
                "wq_t": _weight_tiles(wq[:, esl]),
                "wk_t": _weight_tiles(wk[:, esl]),
                "wv_o": np.ascontiguousarray(wv[:, esl]),
                "wo_o": np.ascontiguousarray(wo[esl, :]),
            }
        )
    return in_maps


def kernel(x, w_router, b_router, wq, wk, wv, wo):
    x = np.asarray(x, np.float32)
    nc = _get_nc(8)
    in_maps = _build_in_maps(
        dict(x=x, w_router=w_router, b_router=b_router, wq=wq, wk=wk, wv=wv, wo=wo)
    )
    res = run_bass_kernel_spmd(nc, in_maps, core_ids=list(range(8)))
    out = x.copy()
    for b in range(B):
        idx = res.results[2 * b]["idx_out"].astype(np.int64)
        y = np.concatenate(
            [res.results[2 * b]["y_out"], res.results[2 * b + 1]["y_out"]], axis=0
        )
        out[b][idx] = y
    return out



# revision 5
# speedup vs baseline: 4.6564x; 4.6564x over previous
"""MoD (mixture-of-depths) attention Bass kernel for Trainium2, 8 NeuronCores.

Problem: B=4, L=4096, D=1024, H=16, HD=64, K=1024 (top-25% tokens per row).
  scores = x @ w_router + b ; idx = top_k(scores, 1024) (desc order)
  xs = x[idx]; causal attention over score-ordered subsequence; out = x with
  selected rows replaced by attention output.

Split of work:
  Host: router scores (fp32 matvec), top-k + descending ordering, gather of
  the K selected rows, transpose/tiling into the exact SBUF layouts (bf16),
  final scatter + pair-sum. These are selection/layout ops — cheap on host,
  expensive on device — and doing them here removes all gpsimd custom ops,
  indirect DMAs and collectives from the device program while cutting the
  staged bytes from ~256MB to ~40MB.

  Device (8 cores, no collectives): core pair (2b, 2b+1) handles batch row b;
  within a pair the 16 heads are split 8/8. Each core runs a dense pipeline in
  bf16 (fp32 PSUM accumulate): V/Q/K projections, causal attention over the
  score-ordered subsequence (S^T tiles [128tk, 512tq], exp on the scalar
  engine, 0/1 causal mask multiply on the vector engine, softmax denominator
  via an extra ones-column in V, normalization via a reciprocal outer-product
  matmul), then a partial out-projection over its 512 e-dims. Host adds the
  two partials and scatters: out[b] = x[b].copy(); out[b][idx] = yA + yB.
"""

import numpy as np
import ml_dtypes

import concourse.bass as bass
import concourse.bacc as bacc
import concourse.mybir as mybir
import concourse.tile as tile
from concourse.bass_utils import run_bass_kernel_spmd

F32 = mybir.dt.float32
F32R = mybir.dt.float32r
BF16 = mybir.dt.bfloat16
AF = mybir.ActivationFunctionType
OP = mybir.AluOpType
BF = ml_dtypes.bfloat16

B, L, D = 4, 4096, 1024
H, HD = 16, 64
K = 1024
SCALE = 1.0 / 8.0
EH = 512          # e-dims per core (8 heads)
N_TC = 8          # token chunks of 128 (K = 1024)
N_EBLK = 4        # e-blocks of 128 per core


def _masks():
    # 0/1 causal masks for S^T tiles [tk=128, tq=512]: tile (m, n) crossing
    # the diagonal has di = m - 4n in {0,1,2,3}; entry (p, f) is valid iff
    # tk <= tq i.e. p + di*128 <= f.
    p = np.arange(128)[:, None]
    f = np.arange(512)[None, :]
    m = np.zeros((4, 128, 512), dtype=BF)
    for di in range(4):
        m[di] = (p + di * 128 <= f).astype(BF)
    return m


def build_program(n_cores=8, percore_shapes=False):
    """Builds the SPMD Bass program (same program on all cores; per-core
    behavior comes only from per-core input data). The program is
    collective-free, so the n_cores=1 build is identical in structure and
    is used for TimelineSim."""
    nc = bacc.Bacc("TRN2", num_devices=n_cores, debug=False)

    # ---- I/O (bf16, pre-tiled host-side into exact SBUF layouts) ----
    # xsT_in[p, dblk, t] = xs[t, dblk*128 + p]
    xsT_in = nc.dram_tensor("xsT_in", [128, 8, K], BF16, kind="ExternalInput")
    # wq_in[p, eblk, dblk, e'] = wq[dblk*128 + p, esl_start + eblk*128 + e']
    wq_in = nc.dram_tensor("wq_in", [128, N_EBLK, 8, 128], BF16, kind="ExternalInput")
    wk_in = nc.dram_tensor("wk_in", [128, N_EBLK, 8, 128], BF16, kind="ExternalInput")
    # wv_in[p, dblk, e] = wv[dblk*128 + p, esl_start + e]
    wv_in = nc.dram_tensor("wv_in", [128, 8, EH], BF16, kind="ExternalInput")
    # wo_in[p, eblk, d] = wo[esl_start + eblk*128 + p, d]
    wo_in = nc.dram_tensor("wo_in", [128, N_EBLK, D], BF16, kind="ExternalInput")
    y_out = nc.dram_tensor("y_out", [K, D], BF16, kind="ExternalOutput")

    m01 = nc.inline_tensor(_masks(), name="c_m01")

    with tile.TileContext(nc) as tc:
        with (
            tc.tile_pool(name="act", bufs=1) as actp,
            tc.tile_pool(name="wts", bufs=1) as wp,
            tc.tile_pool(name="es", bufs=4) as esp,
            tc.tile_pool(name="sm", bufs=2) as smp,
            tc.tile_pool(name="ys", bufs=2) as ysp,
            tc.tile_pool(name="psS", bufs=4, space="PSUM") as psS,
            tc.tile_pool(name="psPO", bufs=2, space="PSUM") as psPO,
            tc.tile_pool(name="psR", bufs=1, space="PSUM") as psR,
        ):
            # ---------- constants + inputs to SBUF ----------
            masks = wp.tile([128, 4, 512], BF16)
            for di in range(4):
                nc.sync.dma_start(masks[:, di, :], m01[di])
            xsT = actp.tile([128, 8, K], BF16)
            nc.sync.dma_start(xsT[:], xsT_in[:])
            wv_sb = wp.tile([128, 8, EH], BF16)
            nc.sync.dma_start(wv_sb[:], wv_in[:])
            wq_sb = wp.tile([128, N_EBLK, 8, 128], BF16)
            nc.sync.dma_start(wq_sb[:], wq_in[:])
            wk_sb = wp.tile([128, N_EBLK, 8, 128], BF16)
            nc.sync.dma_start(wk_sb[:], wk_in[:])
            wo_sb = wp.tile([128, N_EBLK, D], BF16)
            nc.sync.dma_start(wo_sb[:], wo_in[:])

            ones_bf = wp.tile([1, 64], BF16)
            nc.vector.memset(ones_bf[:], 1.0)

            # ---------- V [tc][128t, 8h, 65] (col 64 = ones for denom) ----------
            v_sb = actp.tile([128, N_TC, 8, 65], BF16)
            v_one = wp.tile([128, N_TC * 8], BF16)
            nc.vector.memset(v_one[:], 1.0)
            nc.vector.tensor_copy(
                v_sb[:, :, :, 64], v_one[:].rearrange("p (t h) -> p t h", t=N_TC)
            )
            for t in range(N_TC):
                pv = psS.tile([128, 512], F32, tag="ps")
                for dblk in range(8):
                    nc.tensor.matmul(
                        pv[:],
                        xsT[:, dblk, t * 128 : (t + 1) * 128],
                        wv_sb[:, dblk, :],
                        start=(dblk == 0), stop=(dblk == 7),
                    )
                nc.any.tensor_copy(
                    v_sb[:, t, :, 0:64], pv[:].rearrange("p (h e) -> p h e", h=8)
                )

            # ---------- per eblk: Q^T/K^T [128e, 1024t], then 2 heads' attn ----------
            qT = actp.tile([128, N_EBLK, K], BF16)
            kT = actp.tile([128, N_EBLK, K], BF16)
            oT = actp.tile([128, N_EBLK, K], BF16)

            for eblk in range(N_EBLK):
                for tch in range(2):
                    tsl = bass.ts(tch, 512)
                    pq = psS.tile([128, 512], F32, tag="ps")
                    for dblk in range(8):
                        nc.tensor.matmul(
                            pq[:], wq_sb[:, eblk, dblk, :], xsT[:, dblk, tsl],
                            start=(dblk == 0), stop=(dblk == 7),
                        )
                    nc.any.tensor_copy(qT[:, eblk, tsl], pq[:])
                    pk = psS.tile([128, 512], F32, tag="ps")
                    for dblk in range(8):
                        nc.tensor.matmul(
                            pk[:], wk_sb[:, eblk, dblk, :], xsT[:, dblk, tsl],
                            start=(dblk == 0), stop=(dblk == 7),
                        )
                    nc.any.tensor_copy(kT[:, eblk, tsl], pk[:])

                for sub in range(2):
                    hh = eblk * 2 + sub
                    esl = slice(sub * 64, sub * 64 + 64)
                    for n in range(2):
                        tql = bass.ts(n, 512)
                        po = psPO.tile([65, 512], F32, tag="po")
                        n_m = 4 * n + 4
                        for m in range(n_m):
                            ps_s = psS.tile([128, 512], F32, tag="ps")
                            nc.tensor.matmul(
                                ps_s[:],
                                kT[esl, eblk, m * 128 : (m + 1) * 128],
                                qT[esl, eblk, tql],
                                start=True, stop=True,
                                tile_position=(sub * 64, 0),
                            )
                            es = esp.tile([128, 512], BF16, tag="es")
                            nc.scalar.activation(es[:], ps_s[:], AF.Exp, scale=SCALE)
                            di = m - 4 * n
                            if di >= 0:
                                nc.vector.tensor_tensor(
                                    out=es[:], in0=es[:], in1=masks[:, di, :],
                                    op=OP.mult,
                                )
                            nc.tensor.matmul(
                                po[:], v_sb[:, m, hh, :], es[:],
                                start=(m == 0), stop=(m == n_m - 1),
                            )
                        # normalize rows 0..63 by row 64 (denominator);
                        # bf16 reciprocal (0.4% rounding) is well within the
                        # tolerance budget and keeps the broadcast outer
                        # product at 1 cycle/row.
                        r_row = smp.tile([1, 512], BF16, tag="rr")
                        with nc.allow_low_precision(reason="softmax denom bf16"):
                            nc.vector.reciprocal(r_row[:], po[64:65, :])
                        r_bc = psR.tile([64, 512], F32, tag="rbc")
                        nc.tensor.matmul(
                            r_bc[:], ones_bf[:], r_row[:], start=True, stop=True,
                        )
                        # vector ops may read only one PSUM operand: stage the
                        # broadcast reciprocal in SBUF before the multiply
                        r_sb = smp.tile([64, 512], BF16, tag="rsb")
                        nc.any.tensor_copy(r_sb[:], r_bc[:])
                        nc.vector.tensor_tensor(
                            out=oT[esl, eblk, tql],
                            in0=po[0:64, :], in1=r_sb[:], op=OP.mult,
                        )

            # ---------- out-projection partial: y[t, :] over own 512 e-dims ----------
            for t in range(N_TC):
                for dc in range(2):
                    py = psS.tile([128, 512], F32, tag="ps")
                    for eblk in range(N_EBLK):
                        nc.tensor.matmul(
                            py[:],
                            oT[:, eblk, t * 128 : (t + 1) * 128],
                            wo_sb[:, eblk, dc * 512 : (dc + 1) * 512],
                            start=(eblk == 0), stop=(eblk == N_EBLK - 1),
                        )
                    y_sb = ysp.tile([128, 512], BF16, tag="ysb")
                    nc.any.tensor_copy(y_sb[:], py[:])
                    nc.sync.dma_start(
                        y_out[t * 128 : (t + 1) * 128, dc * 512 : (dc + 1) * 512],
                        y_sb[:],
                    )

    nc.compile()
    return nc


_NC_CACHE = {}


def _get_nc(n_cores=8):
    if n_cores not in _NC_CACHE:
        _NC_CACHE[n_cores] = build_program(n_cores)
    return _NC_CACHE[n_cores]


def _route_and_gather(x, w_router):
    """Host router: top-K indices per row (descending score, ties by index)
    and the gathered rows tiled to the device layout [128, 8dblk, K] bf16."""
    scores = x.reshape(-1, D) @ w_router  # bias shifts all scores equally;
    scores = scores.reshape(B, L)         # it cannot change the top-k or order
    idxs, xsTs = [], []
    for b in range(B):
        s = scores[b]
        part = np.argpartition(-s, K - 1)[:K]
        idx = part[np.lexsort((part, -s[part]))]
        idxs.append(idx)
        xsT = np.ascontiguousarray(x[b][idx].T)          # [D, K]
        xsT = xsT.reshape(8, 128, K).transpose(1, 0, 2)  # [p, dblk, t]
        xsTs.append(np.ascontiguousarray(xsT.astype(BF)))
    return idxs, xsTs


def _prep_weight_half(wq, wk, wv, wo, half):
    esl = slice(half * EH, (half + 1) * EH)
    wqh = wq[:, esl].reshape(8, 128, N_EBLK, 128).transpose(1, 2, 0, 3)
    wkh = wk[:, esl].reshape(8, 128, N_EBLK, 128).transpose(1, 2, 0, 3)
    wvh = wv[:, esl].reshape(8, 128, EH).transpose(1, 0, 2)
    woh = wo[esl, :].reshape(N_EBLK, 128, D).transpose(1, 0, 2)
    return {
        "wq_in": np.ascontiguousarray(wqh.astype(BF)),
        "wk_in": np.ascontiguousarray(wkh.astype(BF)),
        "wv_in": np.ascontiguousarray(wvh.astype(BF)),
        "wo_in": np.ascontiguousarray(woh.astype(BF)),
    }


def kernel(x, w_router, b_router, wq, wk, wv, wo):
    x = np.ascontiguousarray(np.asarray(x, np.float32))
    w_router = np.asarray(w_router, np.float32).reshape(D)
    wq = np.asarray(wq, np.float32)
    wk = np.asarray(wk, np.float32)
    wv = np.asarray(wv, np.float32)
    wo = np.asarray(wo, np.float32)

    idxs, xsTs = _route_and_gather(x, w_router)
    halves = [_prep_weight_half(wq, wk, wv, wo, h) for h in range(2)]
    in_maps = [{"xsT_in": xsTs[c // 2], **halves[c % 2]} for c in range(8)]

    nc = _get_nc(8)
    res = run_bass_kernel_spmd(nc, in_maps, core_ids=list(range(8)))

    out = x.copy()
    for b in range(B):
        ya = res.results[2 * b]["y_out"].astype(np.float32)
        yb = res.results[2 * b + 1]["y_out"].astype(np.float32)
        out[b][idxs[b]] = ya + yb
    return out


# revision 25
# speedup vs baseline: 5.6499x; 1.2134x over previous
"""MoD (mixture-of-depths) attention Bass kernel for Trainium2, 8 NeuronCores.

Problem: B=4, L=4096, D=1024, H=16, HD=64, K=1024 (top-25% tokens per row).
  scores = x @ w_router + b ; idx = top_k(scores, 1024) (desc order)
  xs = x[idx]; causal attention over score-ordered subsequence; out = x with
  selected rows replaced by attention output.

Split of work:
  Host: router scores (fp32 matvec), top-k + descending ordering, gather of
  the K selected rows, transpose/tiling into the exact SBUF layouts (bf16),
  final scatter + pair-sum. These are selection/layout ops — cheap on host,
  expensive on device — and doing them here removes all gpsimd custom ops,
  indirect DMAs and collectives from the device program while cutting the
  staged bytes from ~256MB to ~40MB.

  Device (8 cores, no collectives): core pair (2b, 2b+1) handles batch row b;
  within a pair the 16 heads are split 8/8. Each core runs a dense pipeline in
  bf16 (fp32 PSUM accumulate): V/Q/K projections, causal attention over the
  score-ordered subsequence (S^T tiles [128tk, 512tq], exp on the scalar
  engine, 0/1 causal mask multiply on the vector engine, softmax denominator
  via an extra ones-column in V, normalization via a reciprocal outer-product
  matmul), then a partial out-projection over its 512 e-dims. Host adds the
  two partials and scatters: out[b] = x[b].copy(); out[b][idx] = yA + yB.
"""

import numpy as np
import ml_dtypes

import concourse.bass as bass
import concourse.bacc as bacc
import concourse.mybir as mybir
import concourse.tile as tile

F32 = mybir.dt.float32
F32R = mybir.dt.float32r
BF16 = mybir.dt.bfloat16
AF = mybir.ActivationFunctionType
OP = mybir.AluOpType
BF = ml_dtypes.bfloat16

B, L, D = 4, 4096, 1024
H, HD = 16, 64
K = 1024
SCALE = 1.0 / 8.0
EH = 512          # e-dims per core (8 heads)
N_TC = 8          # token chunks of 128 (K = 1024)
N_EBLK = 4        # e-blocks of 128 per core


def _masks():
    # 0/1 causal masks for S^T tiles [tk=128, tq=512]: tile (m, n) crossing
    # the diagonal has di = m - 4n in {0,1,2,3}; entry (p, f) is valid iff
    # tk <= tq i.e. p + di*128 <= f.
    p = np.arange(128)[:, None]
    f = np.arange(512)[None, :]
    m = np.zeros((4, 128, 512), dtype=BF)
    for di in range(4):
        m[di] = (p + di * 128 <= f).astype(BF)
    return m


def build_program(n_cores=8, percore_shapes=False):
    """Builds the SPMD Bass program (same program on all cores; per-core
    behavior comes only from per-core input data). The program is
    collective-free, so the n_cores=1 build is identical in structure and
    is used for TimelineSim."""
    nc = bacc.Bacc("TRN2", num_devices=n_cores, debug=False)

    # ---- I/O (bf16, pre-tiled host-side into exact SBUF layouts) ----
    # xsT_in[p, dblk, t] = xs[t, dblk*128 + p]
    xsT_in = nc.dram_tensor("xsT_in", [128, 8, K], BF16, kind="ExternalInput")
    # wq_in[p, eblk, dblk, e'] = wq[dblk*128 + p, esl_start + eblk*128 + e']
    wq_in = nc.dram_tensor("wq_in", [128, N_EBLK, 8, 128], BF16, kind="ExternalInput")
    wk_in = nc.dram_tensor("wk_in", [128, N_EBLK, 8, 128], BF16, kind="ExternalInput")
    # wv_in[p, dblk, e] = wv[dblk*128 + p, esl_start + e]
    wv_in = nc.dram_tensor("wv_in", [128, 8, EH], BF16, kind="ExternalInput")
    # wo_in[p, eblk, d] = wo[esl_start + eblk*128 + p, d]
    wo_in = nc.dram_tensor("wo_in", [128, N_EBLK, D], BF16, kind="ExternalInput")
    y_out = nc.dram_tensor("y_out", [K, D], BF16, kind="ExternalOutput")

    m01 = nc.inline_tensor(_masks(), name="c_m01")

    with tile.TileContext(nc) as tc:
        with (
            tc.tile_pool(name="act", bufs=1) as actp,
            tc.tile_pool(name="wts", bufs=1) as wp,
            tc.tile_pool(name="es", bufs=12) as esp,
            tc.tile_pool(name="sm", bufs=2) as smp,
            tc.tile_pool(name="ys", bufs=6) as ysp,
            tc.tile_pool(name="psS", bufs=5, space="PSUM") as psS,
            tc.tile_pool(name="psPO", bufs=2, space="PSUM") as psPO,
            tc.tile_pool(name="psR", bufs=1, space="PSUM") as psR,
        ):
            # ---------- constants + inputs to SBUF ----------
            # order matters: the V phase needs only wv + the first token
            # quarter of xsT, so those transfer first and compute starts
            # ~6us earlier than a monolithic load.
            wv_sb = wp.tile([128, 8, EH], BF16)
            nc.sync.dma_start(wv_sb[:], wv_in[:])
            xsT = actp.tile([128, 8, K], BF16)
            for q in range(4):
                qsl = bass.ts(q, 256)
                nc.sync.dma_start(xsT[:, :, qsl], xsT_in[:, :, qsl])
            wq_sb = wp.tile([128, N_EBLK, 8, 128], BF16)
            nc.sync.dma_start(wq_sb[:], wq_in[:])
            wk_sb = wp.tile([128, N_EBLK, 8, 128], BF16)
            nc.sync.dma_start(wk_sb[:], wk_in[:])
            masks = wp.tile([128, 4, 512], BF16)
            for di in range(4):
                nc.sync.dma_start(masks[:, di, :], m01[di])
            wo_sb = wp.tile([128, N_EBLK, D], BF16)
            nc.sync.dma_start(wo_sb[:], wo_in[:])

            ones_bf = wp.tile([1, 64], BF16)
            nc.vector.memset(ones_bf[:], 1.0)

            # ---------- V [tc][128t, 8h, 65] (col 64 = ones for denom) ----------
            v_sb = actp.tile([128, N_TC, 8, 65], BF16)
            v_one = wp.tile([128, N_TC * 8], BF16)
            nc.vector.memset(v_one[:], 1.0)
            nc.vector.tensor_copy(
                v_sb[:, :, :, 64], v_one[:].rearrange("p (t h) -> p t h", t=N_TC)
            )
            for t in range(N_TC):
                pv = psS.tile([128, 512], F32, tag="ps")
                for dblk in range(8):
                    nc.tensor.matmul(
                        pv[:],
                        xsT[:, dblk, t * 128 : (t + 1) * 128],
                        wv_sb[:, dblk, :],
                        start=(dblk == 0), stop=(dblk == 7),
                    )
                # ACT is idle during the V phase
                nc.scalar.activation(
                    v_sb[:, t, :, 0:64],
                    pv[:].rearrange("p (h e) -> p h e", h=8),
                    AF.Copy,
                )

            # ---------- Q^T/K^T [128e, 1024t] + attention, software-pipelined ----
            qT = actp.tile([128, N_EBLK, K], BF16)
            kT = actp.tile([128, N_EBLK, K], BF16)
            oT = actp.tile([128, N_EBLK, K], BF16)

            def qk_unit(eblk, tch, w_sb, dst):
                # one 8-matmul projection chain + its PSUM->SBUF(bf16) copy
                tsl = bass.ts(tch, 512)
                ps = psS.tile([128, 512], F32, tag="ps")
                for dblk in range(8):
                    nc.tensor.matmul(
                        ps[:], w_sb[:, eblk, dblk, :], xsT[:, dblk, tsl],
                        start=(dblk == 0), stop=(dblk == 7),
                    )
                nc.vector.tensor_copy(dst[:, eblk, tsl], ps[:])

            def qk_units(eblk):
                return [
                    (eblk, tch, w, d)
                    for tch in range(2)
                    for (w, d) in ((wq_sb, qT), (wk_sb, kT))
                ]

            # spread DMAs across engine queues so DGE setup parallelizes
            dma_queues = [nc.sync, nc.scalar]

            def outproj_unit(t, dc):
                py = psS.tile([128, 512], F32, tag="ps")
                for eblk in range(N_EBLK):
                    nc.tensor.matmul(
                        py[:],
                        oT[:, eblk, t * 128 : (t + 1) * 128],
                        wo_sb[:, eblk, dc * 512 : (dc + 1) * 512],
                        start=(eblk == 0), stop=(eblk == N_EBLK - 1),
                    )
                y_sb = ysp.tile([128, 512], BF16, tag="ysb")
                nc.vector.tensor_copy(y_sb[:], py[:])
                dma_queues[(2 * t + dc) % 2].dma_start(
                    y_out[t * 128 : (t + 1) * 128, dc * 512 : (dc + 1) * 512],
                    y_sb[:],
                )

            # queue of independent PE chain emitters, pumped mid-attention so
            # the PE always has work while exp/mask streams catch up
            filler = []

            def pump():
                if filler:
                    filler.pop(0)()

            # normalization for a finished block is deferred into the NEXT
            # block so its reciprocal latency never blocks the PE stream
            pending_norm = []

            def normalize(eblk, sub, n, po):
                esl = slice(sub * 64, sub * 64 + 64)
                tql = bass.ts(n, 512)
                # bf16 reciprocal (0.4% rounding) is well within the
                # tolerance budget and keeps the broadcast outer product at
                # 1 cycle/row.
                r_row = smp.tile([1, 512], BF16, tag="rr")
                with nc.allow_low_precision(reason="softmax denom bf16"):
                    nc.vector.reciprocal(r_row[:], po[64:65, :])
                r_bc = psR.tile([64, 512], F32, tag="rbc")
                nc.tensor.matmul(
                    r_bc[:], ones_bf[:], r_row[:], start=True, stop=True,
                )
                # vector ops may read only one PSUM operand: stage the
                # broadcast reciprocal in SBUF before the multiply
                r_sb = smp.tile([64, 512], BF16, tag="rsb")
                nc.vector.tensor_copy(r_sb[:], r_bc[:])
                nc.vector.tensor_tensor(
                    out=oT[esl, eblk, tql],
                    in0=po[0:64, :], in1=r_sb[:], op=OP.mult,
                )

            def flush_norm():
                while pending_norm:
                    pending_norm.pop(0)()

            def attn_block(eblk, sub, n):
                hh = eblk * 2 + sub
                esl = slice(sub * 64, sub * 64 + 64)
                po = psPO.tile([65, 512], F32, tag="po")
                n_m = 4 * n + 4
                es_tiles = []

                def s_tile(m):
                    # a diagonal-crossing tile (di >= 0) has its first di*128
                    # columns fully masked for every partition: restrict
                    # S/exp/mask/PV to the live columns.
                    di = m - 4 * n
                    lo = di * 128 if di > 0 else 0
                    csl = slice(lo, 512)
                    ps_s = psS.tile([128, 512], F32, tag="ps")
                    nc.tensor.matmul(
                        ps_s[:, csl],
                        kT[esl, eblk, m * 128 : (m + 1) * 128],
                        qT[esl, eblk, n * 512 + lo : (n + 1) * 512],
                        start=True, stop=True,
                        tile_position=(sub * 64, 0),
                    )
                    es = esp.tile([128, 512], BF16, tag="es")
                    nc.scalar.activation(
                        es[:, csl], ps_s[:, csl], AF.Exp, scale=SCALE
                    )
                    if di >= 0:
                        # SBUF-only multiply: wide masks go to the otherwise-
                        # idle gpsimd engine, narrow ones to the vector engine
                        eng = nc.gpsimd if di < 2 else nc.vector
                        eng.tensor_tensor(
                            out=es[:, csl], in0=es[:, csl],
                            in1=masks[:, di, csl], op=OP.mult,
                        )
                    es_tiles.append((es, csl))

                def pv_tile(m):
                    es, csl = es_tiles[m]
                    nc.tensor.matmul(
                        po[:, csl], v_sb[:, m, hh, :], es[:, csl],
                        start=(m == 0), stop=(m == n_m - 1),
                    )

                # S runs ~4 tiles ahead of PV so the PE never waits for the
                # exp/mask stream; the pump slots independent chain work in
                # the middle of the block.
                for m in range(4):
                    s_tile(m)
                flush_norm()
                pump()
                for m in range(4, n_m):
                    s_tile(m)
                    pv_tile(m - 4)
                for m in range(max(n_m - 4, 0), n_m):
                    pv_tile(m)
                pump()
                pending_norm.append(lambda: normalize(eblk, sub, n, po))

            for u in qk_units(0):
                qk_unit(*u)

            for eblk in range(N_EBLK - 1):
                filler.extend(
                    (lambda u=u: qk_unit(*u)) for u in qk_units(eblk + 1)
                )
                for sub in range(2):
                    for n in range(2):
                        attn_block(eblk, sub, n)

            # last eblk: run both heads' n=0 blocks first so the t<512
            # out-projection columns unlock early, then interleave those
            # out-proj chains into the n=1 blocks.
            emitted = []

            def op_filler(t, dc):
                emitted.append((t, dc))
                outproj_unit(t, dc)

            attn_block(N_EBLK - 1, 0, 0)
            attn_block(N_EBLK - 1, 1, 0)
            filler.extend(
                (lambda t=t, dc=dc: op_filler(t, dc))
                for t in (0, 1) for dc in (0, 1)
            )
            attn_block(N_EBLK - 1, 0, 1)
            attn_block(N_EBLK - 1, 1, 1)
            flush_norm()

            # ---------- remaining out-projection partials ----------
            for t in range(N_TC):
                for dc in range(2):
                    if (t, dc) not in emitted:
                        outproj_unit(t, dc)

    nc.compile()
    return nc


_NC_CACHE = {}


def _get_nc(n_cores=8):
    if n_cores not in _NC_CACHE:
        _NC_CACHE[n_cores] = build_program(n_cores)
    return _NC_CACHE[n_cores]


_DISPATCH_CACHE = {}


def _get_dispatch(nc, n_cores=8):
    """SPMD dispatch tuned for the axon tunnel: per-device parallel
    device_put of the inputs (~3x the single-stream tunnel bandwidth),
    output donation buffers created on-device instead of uploading zeros,
    and a jit cached across kernel() calls. Mirrors the metadata handling
    of bass2jax.run_bass_via_pjrt, minus collectives/partition-id support
    (this program uses neither)."""
    key = id(nc)
    if key in _DISPATCH_CACHE:
        return _DISPATCH_CACHE[key]

    import jax
    import jax.numpy as jnp
    from jax.sharding import Mesh, NamedSharding, PartitionSpec
    from jax.experimental.shard_map import shard_map
    from concourse.bass2jax import (
        _bass_exec_p,
        install_neuronx_cc_hook,
        partition_id_tensor,
    )

    install_neuronx_cc_hook()
    assert nc.dbg_addr is None

    partition_name = (
        nc.partition_id_tensor.name if nc.partition_id_tensor else None
    )
    in_names, out_names, out_avals = [], [], []
    for alloc in nc.m.functions[0].allocations:
        if not isinstance(alloc, mybir.MemoryLocationSet):
            continue
        name = alloc.memorylocations[0].name
        if alloc.kind == "ExternalInput":
            if name != partition_name:
                in_names.append(name)
        elif alloc.kind == "ExternalOutput":
            out_names.append(name)
            out_avals.append(
                jax.core.ShapedArray(
                    tuple(alloc.tensor_shape), mybir.dt.np(alloc.dtype)
                )
            )
    n_params = len(in_names)
    all_names = list(in_names) + list(out_names)
    if partition_name is not None:
        all_names.append(partition_name)
    all_names = tuple(all_names)

    def _body(*args):
        operands = list(args)
        if partition_name is not None:
            operands.append(partition_id_tensor())
        return tuple(
            _bass_exec_p.bind(
                *operands,
                out_avals=tuple(out_avals),
                in_names=all_names,
                out_names=tuple(out_names),
                lowering_input_output_aliases=(),
                sim_require_finite=True,
                sim_require_nnan=True,
                nc=nc,
            )
        )

    devices = jax.devices()[:n_cores]
    mesh = Mesh(np.asarray(devices), ("core",))
    spec = NamedSharding(mesh, PartitionSpec("core"))
    donate = tuple(range(n_params, n_params + len(out_avals)))
    sharded = jax.jit(
        shard_map(
            _body,
            mesh=mesh,
            in_specs=(PartitionSpec("core"),) * (n_params + len(out_avals)),
            out_specs=(PartitionSpec("core"),) * len(out_names),
            check_rep=False,
        ),
        donate_argnums=donate,
        keep_unused=True,
    )

    def dispatch(in_maps):
        # parallel H2D: put every core's shard of every input, then stitch
        # the global arrays from the device-resident shards.
        shards = [
            [jax.device_put(np.asarray(in_maps[c][n]), devices[c]) for c in range(n_cores)]
            for n in in_names
        ]
        global_in = []
        for i, n in enumerate(in_names):
            sh = shards[i][0].shape
            global_in.append(
                jax.make_array_from_single_device_arrays(
                    (n_cores * sh[0], *sh[1:]), spec, shards[i]
                )
            )
        # donated output buffers: created on-device, nothing uploaded
        zero_shards = []
        for av in out_avals:
            zs = []
            for d in devices:
                with jax.default_device(d):
                    zs.append(jnp.zeros(av.shape, av.dtype))
            jax.block_until_ready(zs)
            zero_shards.append(
                jax.make_array_from_single_device_arrays(
                    (n_cores * av.shape[0], *av.shape[1:]), spec, zs
                )
            )
        out_arrs = sharded(*global_in, *zero_shards)
        out_np = [np.asarray(a) for a in out_arrs]
        return [
            {
                name: out_np[i].reshape(n_cores, *out_avals[i].shape)[c]
                for i, name in enumerate(out_names)
            }
            for c in range(n_cores)
        ]

    _DISPATCH_CACHE[key] = dispatch
    return dispatch


def _route_and_gather(x, w_router):
    """Host router: top-K indices per row (descending score, ties by index)
    and the gathered rows tiled to the device layout [128, 8dblk, K] bf16."""
    scores = x.reshape(-1, D) @ w_router  # bias shifts all scores equally;
    scores = scores.reshape(B, L)         # it cannot change the top-k or order
    idxs, xsTs = [], []
    for b in range(B):
        s = scores[b]
        part = np.argpartition(-s, K - 1)[:K]
        idx = part[np.lexsort((part, -s[part]))]
        idxs.append(idx)
        xsT = np.ascontiguousarray(x[b][idx].T)          # [D, K]
        xsT = xsT.reshape(8, 128, K).transpose(1, 0, 2)  # [p, dblk, t]
        xsTs.append(np.ascontiguousarray(xsT.astype(BF)))
    return idxs, xsTs


def _prep_weight_half(wq, wk, wv, wo, half):
    esl = slice(half * EH, (half + 1) * EH)
    wqh = wq[:, esl].reshape(8, 128, N_EBLK, 128).transpose(1, 2, 0, 3)
    wkh = wk[:, esl].reshape(8, 128, N_EBLK, 128).transpose(1, 2, 0, 3)
    wvh = wv[:, esl].reshape(8, 128, EH).transpose(1, 0, 2)
    woh = wo[esl, :].reshape(N_EBLK, 128, D).transpose(1, 0, 2)
    return {
        "wq_in": np.ascontiguousarray(wqh.astype(BF)),
        "wk_in": np.ascontiguousarray(wkh.astype(BF)),
        "wv_in": np.ascontiguousarray(wvh.astype(BF)),
        "wo_in": np.ascontiguousarray(woh.astype(BF)),
    }


def kernel(x, w_router, b_router, wq, wk, wv, wo):
    x = np.ascontiguousarray(np.asarray(x, np.float32))
    w_router = np.asarray(w_router, np.float32).reshape(D)
    wq = np.asarray(wq, np.float32)
    wk = np.asarray(wk, np.float32)
    wv = np.asarray(wv, np.float32)
    wo = np.asarray(wo, np.float32)

    idxs, xsTs = _route_and_gather(x, w_router)
    halves = [_prep_weight_half(wq, wk, wv, wo, h) for h in range(2)]
    in_maps = [{"xsT_in": xsTs[c // 2], **halves[c % 2]} for c in range(8)]

    nc = _get_nc(8)
    results = _get_dispatch(nc, 8)(in_maps)

    out = x.copy()
    for b in range(B):
        ya = results[2 * b]["y_out"].astype(np.float32)
        yb = results[2 * b + 1]["y_out"].astype(np.float32)
        out[b][idxs[b]] = ya + yb
    return out
